# revision 2
# baseline (speedup 1.0000x reference)
"""Trainium2 Bass kernel for nn_NeighbourAssignment.

Math (per edge e with target node n=idx[e]):
    logits = own @ Wt + src @ Ws + (bt + bs)          (E, 4)
    a      = softmax(logits)                          (E, 4)
    out[s, n, :] = (sum_e a[e,s] * (src[e] @ W_bank[s] + b_bank[s])) / max(cnt[n], 1)

Key algebraic restructure: the segment-sum commutes with the per-edge GEMM:
    G_s[n, :]  = sum_{e->n} a[e,s] * src[e, :]        (N, CS)   <- scatter, on PE
    m_s[n]     = sum_{e->n} a[e,s]                    (N,)
    out[s,n,:] = (G_s[n] @ W_bank[s] + m_s[n] * b_bank[s]) / cnt
This cuts FLOPs ~16x vs materializing (SIZE, E, OUT).

Sharding: edges sorted by node; nodes split evenly across 8 cores
(node-contiguous, so each core owns a disjoint node range and there is no
cross-core reduction; host just concatenates node slices).

Scatter on PE: edges packed into 32-node windows; per 128-edge tile a
(128e x 128) stationary matrix A_cat = [a_0*onehot | a_1*onehot | a_2*onehot |
a_3*onehot] (onehot: edge -> node-within-window) multiplies the (128e, 256c)
src tile, accumulating G for all 4 banks in one matmul stream.
"""
import os
from contextlib import ExitStack

import numpy as np

P = 128
WIN = 32  # nodes per scatter window
SIZE = 4


def _build_plan(idx, n_nodes, n_cores):
    """Sort edges by node, shard nodes across cores, pack into windows.

    Returns dict with per-core edge permutation + padded layout params.
    """
    E = idx.shape[0]
    nodes_per_core = -(-n_nodes // n_cores)  # ceil
    n_pad = -(-nodes_per_core // P) * P  # per-core padded node count
    W = n_pad // WIN  # windows per core

    order = np.argsort(idx, kind="stable")
    sidx = idx[order]
    core_of = np.minimum(sidx // nodes_per_core, n_cores - 1)
    core_starts = np.searchsorted(core_of, np.arange(n_cores + 1))

    local = sidx - core_of * nodes_per_core  # local node id per sorted edge
    win_of = local // WIN

    # T = max tiles needed by any (core, window)
    T = 1
    win_counts = []
    for c in range(n_cores):
        lo, hi = core_starts[c], core_starts[c + 1]
        cnts = np.bincount(win_of[lo:hi], minlength=W)
        win_counts.append(cnts)
        if hi > lo:
            T = max(T, int(-(-cnts.max() // P)))
    E_pad = W * T * P

    plans = []
    for c in range(n_cores):
        lo, hi = core_starts[c], core_starts[c + 1]
        cnts = win_counts[c]
        # position of each edge inside the padded layout
        perm = order[lo:hi]  # global edge ids, sorted by local node
        o_reb = np.full(E_pad, -1.0, dtype=np.float32)
        dst = np.empty(hi - lo, dtype=np.int64)
        off = 0
        for w in range(W):
            cw = int(cnts[w])
            dst[off : off + cw] = w * T * P + np.arange(cw)
            off += cw
        o_reb[dst] = (local[lo:hi] - win_of[lo:hi] * WIN).astype(np.float32)
        plans.append({"perm": perm, "dst": dst})
    return {
        "plans": plans,
        "nodes_per_core": nodes_per_core,
        "n_pad": n_pad,
        "W": W,
        "T": T,
        "E_pad": E_pad,
        "o_reb": None,  # filled per core below
        "core_starts": core_starts,
        "order": order,
        "local": local,
        "win_of": win_of,
    }


def _host_prep(own, src, Wt, bt, Ws, bs, W_bank, b_bank, idx, n_nodes, n_cores):
    E = idx.shape[0]
    plan = _build_plan(idx, n_nodes, n_cores)
    npc, n_pad, W, T, E_pad = (
        plan["nodes_per_core"], plan["n_pad"], plan["W"], plan["T"], plan["E_pad"],
    )
    n_tiles = E_pad // P
    counts = np.bincount(idx, minlength=n_nodes).astype(np.float32)
    inv_full = 1.0 / np.maximum(counts, 1.0)

    # logits weight pack: [Wt_c0 | Wt_c1 | Ws_c0 | Ws_c1], each (128, 4)
    Wlog = np.concatenate(
        [Wt[0:P], Wt[P : 2 * P], Ws[0:P], Ws[P : 2 * P]], axis=1
    ).astype(np.float32)  # (128, 16)
    b_row = (bt + bs).astype(np.float32)[None, :]  # (1, 4)
    # final GEMM weights: per (s, chunk): W_bank[s, ch*128:(ch+1)*128, :] (128, 256)
    Wb = np.concatenate(
        [W_bank[s, ch * P : (ch + 1) * P, :] for s in range(SIZE) for ch in range(2)],
        axis=1,
    ).astype(np.float32)  # (128, 2048)
    bb = np.tile(b_bank.reshape(1, SIZE * 256), (P, 1)).astype(np.float32)  # (128,1024)

    in_maps = []
    core_starts, order, local, win_of = (
        plan["core_starts"], plan["order"], plan["local"], plan["win_of"],
    )
    for c in range(n_cores):
        p = plan["plans"][c]
        perm, dst = p["perm"], p["dst"]
        lo, hi = core_starts[c], core_starts[c + 1]

        src_pad = np.zeros((E_pad, 256), dtype=np.float32)
        src_pad[dst] = src[perm]
        own_pad = np.zeros((E_pad, 256), dtype=np.float32)
        own_pad[dst] = own[perm]
        o_reb = np.full(E_pad, -1.0, dtype=np.float32)
        o_reb[dst] = (local[lo:hi] - win_of[lo:hi] * WIN).astype(np.float32)

        inv_local = np.ones(n_pad, dtype=np.float32)
        nreal = min(npc, n_nodes - c * npc)
        inv_local[:nreal] = inv_full[c * npc : c * npc + nreal]
        # invW: (128 = 4s x 32j, W): inv for node 32w + j  (same for each s)
        j = np.arange(P) % WIN
        invW = inv_local[(np.arange(W)[None, :] * WIN) + j[:, None]].astype(np.float32)
        # invG: (128, n_pad//128): inv for node 128g + p
        invG = inv_local.reshape(n_pad // P, P).T.copy()

        in_maps.append({
            "srcD": src_pad,
            "ownTD": np.ascontiguousarray(own_pad.T),
            "srcTD": np.ascontiguousarray(src_pad.T),
            "oD": np.ascontiguousarray(o_reb.reshape(n_tiles, P).T),
            "WlogD": Wlog,
            "browD": b_row,
            "WbD": Wb,
            "bbD": bb,
            "invWD": invW,
            "invGD": invG,
        })
    return plan, in_maps


def _build_program(E_pad, n_pad, W, T):
    import concourse.bacc as bacc
    import concourse.tile as tile
    from concourse import mybir
    from concourse.masks import make_identity

    dt = mybir.dt
    f32 = dt.float32
    AF = mybir.ActivationFunctionType
    OP = mybir.AluOpType

    n_tiles = E_pad // P
    assert n_tiles % 4 == 0
    n_blocks = n_tiles // 4
    G = n_pad // P  # node groups of 128

    nc = bacc.Bacc("TRN2", target_bir_lowering=False, debug=False)

    srcD = nc.dram_tensor("srcD", [E_pad, 256], f32, kind="ExternalInput").ap()
    ownTD = nc.dram_tensor("ownTD", [256, E_pad], f32, kind="ExternalInput").ap()
    srcTD = nc.dram_tensor("srcTD", [256, E_pad], f32, kind="ExternalInput").ap()
    oD = nc.dram_tensor("oD", [P, n_tiles], f32, kind="ExternalInput").ap()
    WlogD = nc.dram_tensor("WlogD", [P, 16], f32, kind="ExternalInput").ap()
    browD = nc.dram_tensor("browD", [1, 4], f32, kind="ExternalInput").ap()
    WbD = nc.dram_tensor("WbD", [P, 2048], f32, kind="ExternalInput").ap()
    bbD = nc.dram_tensor("bbD", [P, 1024], f32, kind="ExternalInput").ap()
    invWD = nc.dram_tensor("invWD", [P, W], f32, kind="ExternalInput").ap()
    invGD = nc.dram_tensor("invGD", [P, G], f32, kind="ExternalInput").ap()
    outD = nc.dram_tensor("outD", [SIZE, n_pad, 256], f32, kind="ExternalOutput").ap()

    with tile.TileContext(nc) as tc, ExitStack() as ctx:
        cst = ctx.enter_context(tc.tile_pool(name="cst", bufs=1))
        lg_in = ctx.enter_context(tc.tile_pool(name="lg_in", bufs=3))
        sm = ctx.enter_context(tc.tile_pool(name="sm", bufs=4))
        srcp = ctx.enter_context(tc.tile_pool(name="srcp", bufs=6))
        ap_ = ctx.enter_context(tc.tile_pool(name="ap", bufs=6))
        gsb = ctx.enter_context(tc.tile_pool(name="gsb", bufs=3))
        outp = ctx.enter_context(tc.tile_pool(name="outp", bufs=3))
        # psum pools (8 banks total: 2+2+2+2)
        ps_a = ctx.enter_context(tc.tile_pool(name="ps_a", bufs=2, space="PSUM"))
        ps_g = ctx.enter_context(tc.tile_pool(name="ps_g", bufs=2, space="PSUM"))
        ps_m = ctx.enter_context(tc.tile_pool(name="ps_m", bufs=2, space="PSUM"))
        ps_t = ctx.enter_context(tc.tile_pool(name="ps_t", bufs=2, space="PSUM"))

        # ---- constants ----
        iota32 = cst.tile([P, WIN], f32, tag="iota32")
        nc.gpsimd.iota(iota32[:], pattern=[[1, WIN]], base=0, channel_multiplier=0,
                       allow_small_or_imprecise_dtypes=True)
        ident = cst.tile([P, P], f32, tag="ident")
        make_identity(nc, ident[:])
        ones_row = cst.tile([1, P], f32, tag="ones_row")
        nc.gpsimd.memset(ones_row[:], 1.0)
        Wlog = cst.tile([P, 16], f32, tag="Wlog")
        nc.sync.dma_start(Wlog[:], WlogD[:])
        brow = cst.tile([1, 4], f32, tag="brow")
        nc.sync.dma_start(brow[:], browD[:])
        Wb = cst.tile([P, 2048], f32, tag="Wb")
        nc.sync.dma_start(Wb[:], WbD[:])
        bb = cst.tile([P, 1024], f32, tag="bb")
        nc.sync.dma_start(bb[:], bbD[:])
        invW = cst.tile([P, W], f32, tag="invW")
        nc.sync.dma_start(invW[:], invWD[:])
        invG = cst.tile([P, G], f32, tag="invG")
        nc.sync.dma_start(invG[:], invGD[:])
        o_all = cst.tile([P, n_tiles], f32, tag="o_all")
        nc.sync.dma_start(o_all[:], oD[:])
        a_all = cst.tile([P, 4 * n_tiles], f32, tag="a_all")
        # persistent outputs of stage B->C
        gtA = cst.tile([P, SIZE * n_pad], f32, tag="gtA")  # G^T chunk0 (c 0:128)
        gtB = cst.tile([P, SIZE * n_pad], f32, tag="gtB")  # G^T chunk1 (c 128:256)
        mT = cst.tile([4, n_pad], f32, tag="mT")

        gtA_v = gtA[:].rearrange("p (s n) -> p s n", s=SIZE)
        gtB_v = gtB[:].rearrange("p (s n) -> p s n", s=SIZE)

        # ---- stage A: logits + softmax -> a_all ----
        for b in range(n_blocks):
            e0 = b * 512
            ownT0 = lg_in.tile([P, 512], f32, tag="ownT0")
            ownT1 = lg_in.tile([P, 512], f32, tag="ownT1")
            srcT0 = lg_in.tile([P, 512], f32, tag="srcT0")
            srcT1 = lg_in.tile([P, 512], f32, tag="srcT1")
            nc.sync.dma_start(ownT0[:], ownTD[0:P, e0 : e0 + 512])
            nc.sync.dma_start(ownT1[:], ownTD[P : 2 * P, e0 : e0 + 512])
            nc.sync.dma_start(srcT0[:], srcTD[0:P, e0 : e0 + 512])
            nc.sync.dma_start(srcT1[:], srcTD[P : 2 * P, e0 : e0 + 512])

            lg = ps_a.tile([P, 16], f32, tag="lg")
            for ti in range(4):
                sl = slice(128 * ti, 128 * ti + 128)
                co = slice(4 * ti, 4 * ti + 4)
                nc.tensor.matmul(lg[:, co], lhsT=ownT0[:, sl], rhs=Wlog[:, 0:4],
                                 start=True, stop=False)
                nc.tensor.matmul(lg[:, co], lhsT=ownT1[:, sl], rhs=Wlog[:, 4:8],
                                 start=False, stop=False)
                nc.tensor.matmul(lg[:, co], lhsT=srcT0[:, sl], rhs=Wlog[:, 8:12],
                                 start=False, stop=False)
                nc.tensor.matmul(lg[:, co], lhsT=srcT1[:, sl], rhs=Wlog[:, 12:16],
                                 start=False, stop=False)
                nc.tensor.matmul(lg[:, co], lhsT=ones_row[:], rhs=brow[:],
                                 start=False, stop=True)

            expt = sm.tile([P, 16], f32, tag="expt")
            nc.scalar.activation(expt[:], lg[:], AF.Exp)
            Z = sm.tile([P, 4], f32, tag="Z")
            nc.vector.tensor_reduce(
                Z[:], expt[:].rearrange("p (t s) -> p t s", s=4),
                axis=mybir.AxisListType.X, op=OP.add,
            )
            rZ = sm.tile([P, 4], f32, tag="rZ")
            nc.vector.reciprocal(rZ[:], Z[:])
            for ti in range(4):
                co = slice(4 * ti, 4 * ti + 4)
                nc.vector.tensor_scalar(
                    out=a_all[:, 16 * b + 4 * ti : 16 * b + 4 * ti + 4],
                    in0=expt[:, co], scalar1=rZ[:, ti : ti + 1], scalar2=None,
                    op0=OP.mult,
                )

        # ---- stage B: scatter into G (per window) + mT ----
        for w in range(W):
            g_ps = ps_g.tile([P, 256], f32, tag="g_ps")
            mT_ps = ps_m.tile([4, WIN], f32, tag="mT_ps")
            for t in range(T):
                g = w * T + t
                src_t = srcp.tile([P, 256], f32, tag="src_t")
                nc.sync.dma_start(src_t[:], srcD[P * g : P * g + P, :])
                oh = ap_.tile([P, WIN], f32, tag="oh")
                nc.vector.tensor_scalar(
                    out=oh[:], in0=iota32[:], scalar1=o_all[:, g : g + 1],
                    scalar2=None, op0=OP.is_equal,
                )
                A_cat = ap_.tile([P, P], f32, tag="A_cat")
                for s in range(SIZE):
                    nc.vector.tensor_scalar(
                        out=A_cat[:, WIN * s : WIN * s + WIN], in0=oh[:],
                        scalar1=a_all[:, 4 * g + s : 4 * g + s + 1], scalar2=None,
                        op0=OP.mult,
                    )
                nc.tensor.matmul(g_ps[:], lhsT=A_cat[:], rhs=src_t[:],
                                 start=(t == 0), stop=(t == T - 1))
                nc.tensor.matmul(mT_ps[:], lhsT=a_all[:, 4 * g : 4 * g + 4],
                                 rhs=oh[:], start=(t == 0), stop=(t == T - 1))
            # evacuate: G scaled by 1/count; mT plain
            g_sb = gsb.tile([P, 256], f32, tag="g_sb")
            nc.scalar.activation(g_sb[:], g_ps[:], AF.Copy, bias=0.0,
                                 scale=invW[:, w : w + 1])
            nc.scalar.copy(mT[0:4, WIN * w : WIN * w + WIN], mT_ps[:])
            # transpose the two 128-col chunks; de-interleave into gtA/gtB
            for ch, gt_v in ((0, gtA_v), (1, gtB_v)):
                tp = ps_t.tile([P, P], f32, tag="tp")
                nc.tensor.transpose(tp[:], g_sb[:, 128 * ch : 128 * ch + 128],
                                    ident[:])
                nc.scalar.copy(
                    gt_v[:, :, WIN * w : WIN * w + WIN],
                    tp[:].rearrange("p (s j) -> p s j", s=SIZE),
                )

        # ---- stage C: final GEMM + bias + writeback ----
        for g in range(G):
            mnp = ps_m.tile([P, 4], f32, tag="mT_ps")  # share slots with mT_ps
            nc.tensor.transpose(mnp[:], mT[0:4, P * g : P * g + P], ident[0:4, 0:4])
            m_sb = sm.tile([P, 4], f32, tag="m_sb")
            nc.vector.tensor_scalar(out=m_sb[:], in0=mnp[:],
                                    scalar1=invG[:, g : g + 1], scalar2=None,
                                    op0=OP.mult)
            for s in range(SIZE):
                o_ps = ps_a.tile([P, 256], f32, tag="lg")  # share slots with lg
                nc.tensor.matmul(o_ps[:], lhsT=gtA_v[:, s, P * g : P * g + P],
                                 rhs=Wb[:, (2 * s) * 256 : (2 * s) * 256 + 256],
                                 start=True, stop=False)
                nc.tensor.matmul(o_ps[:], lhsT=gtB_v[:, s, P * g : P * g + P],
                                 rhs=Wb[:, (2 * s + 1) * 256 : (2 * s + 1) * 256 + 256],
                                 start=False, stop=True)
                o_sb = outp.tile([P, 256], f32, tag="o_sb")
                nc.vector.scalar_tensor_tensor(
                    out=o_sb[:], in0=bb[:, 256 * s : 256 * s + 256],
                    scalar=m_sb[:, s : s + 1], in1=o_ps[:],
                    op0=OP.mult, op1=OP.add,
                )
                nc.sync.dma_start(outD[s, P * g : P * g + P, :], o_sb[:])

    nc.compile()
    return nc


_PROG_CACHE = {}


def kernel(own_data, source_message, Wt, bt, Ws_assign, bs_assign,
           W_bank, b_bank, indices, node_count, _trace=False):
    from concourse.bass_utils import run_bass_kernel_spmd

    own = np.asarray(own_data, dtype=np.float32)
    src = np.asarray(source_message, dtype=np.float32)
    Wt = np.asarray(Wt, dtype=np.float32)
    bt = np.asarray(bt, dtype=np.float32)
    Ws = np.asarray(Ws_assign, dtype=np.float32)
    bs = np.asarray(bs_assign, dtype=np.float32)
    W_bank = np.asarray(W_bank, dtype=np.float32)
    b_bank = np.asarray(b_bank, dtype=np.float32)
    idx = np.asarray(indices).astype(np.int64)
    N = int(node_count)
    n_cores = 8

    plan, in_maps = _host_prep(own, src, Wt, bt, Ws, bs, W_bank, b_bank,
                               idx, N, n_cores)
    key = (plan["E_pad"], plan["n_pad"], plan["W"], plan["T"])
    if key not in _PROG_CACHE:
        _PROG_CACHE[key] = _build_program(*key)
    nc = _PROG_CACHE[key]

    res = run_bass_kernel_spmd(nc, in_maps, core_ids=list(range(n_cores)),
                               trace=_trace)
    npc = plan["nodes_per_core"]
    out = np.empty((SIZE, N, 256), dtype=np.float32)
    for c in range(n_cores):
        nreal = min(npc, N - c * npc)
        out[:, c * npc : c * npc + nreal, :] = res.results[c]["outD"][:, :nreal, :]
    if _trace and res.exec_time_ns is not None:
        print(f"HW exec time: {res.exec_time_ns} ns")
    kernel._last_result = res
    return out
